# revision 15
# baseline (speedup 1.0000x reference)
"""Trainium2 Bass kernel for DequantingLinear (GGML Q8_0 dequant + linear).

Computes out[4096, 12288] = x[4096, 3072] @ dequant(w_q, w_scales).T + bias
where w_q is int32 (int8-valued) with per-32-element-block fp32 scales.

Sharding: tensor-parallel over output features across 8 NeuronCores. Each
core gets the full x and a 1536-row shard of w_q / w_scales / bias,
computes its [4096, 1536] output slice; the host concatenates on axis 1.

Per-core kernel (Tile framework), v6:
  * x is staged host-side as xT [3072, 4096] bf16 (pure layout/container
    change; the baseline cast fp32->bf16 in the DGE anyway). Device DMA
    loads [128, 24, 128] x-tiles straight into the transposed GEMM layout
    - zero PE transposes for x (was 768 transposes ~59us of PE time).
  * w path: w_q int32 row-chunks via SWDGE, DVE dequant (int32 x
    block-broadcast fp32 scales -> bf16, exact for |q|<=127), PE
    transposes into the resident wt[in-part, k, out] tensor.
  * GEMM: psum[128 tok, 512 out] accumulates 24 bf16 k-tile matmuls,
    k-inner/n-inner so 3 MMs share each stationary x-tile; a
    post-compile pass drops the redundant LDWEIGHTS the legalizer
    emits per-matmul (~78ns each). Bias is added during the
    PSUM->SBUF copy on the vector engine.
  * Phase-1 (n=0 for the first two token blocks) is interleaved between
    w-chunk groups so the PE never waits for the full w stream.
  HBM traffic/core: x 25.2 MB + w_q 18.9 MB + out 25.2 MB (+0.6).
"""

import os
import sys

for _p in ("/opt/trn_rl_repo",):
    if _p not in sys.path:
        sys.path.append(_p)

from contextlib import ExitStack

import numpy as np

import concourse.bacc as bacc
import concourse.bass as bass
import concourse.mybir as mybir
from concourse import tile
from concourse.tile_rust import add_dep_helper
from concourse.bass_utils import run_bass_kernel_spmd

FP32 = mybir.dt.float32
BF16 = mybir.dt.bfloat16
INT32 = mybir.dt.int32
INT8 = mybir.dt.int8

N_CORES = 8
TOK, IN, OUT = 4096, 3072, 12288
QK = 32
OUT_SH = OUT // N_CORES
NCOL = 512
NB1 = 2  # phase-1 token blocks (of 512 tokens each)


def _build(nc: bass.Bass, repeats: int = 1, serialize: bool = False, mode: str = "full"):
    P = 128
    KT = IN // P          # 24 k-tiles
    NBLK = TOK // 512     # 8 token blocks
    MT = 512 // P         # 4 m-tiles per block
    NT = OUT_SH // NCOL   # 3 n-groups
    NB = IN // QK         # 96 scale blocks
    OT = OUT_SH // P      # 12 w chunks
    TB = 4                # transposes batched per PSUM tile

    xt_d = nc.dram_tensor("xt", [IN, TOK], BF16, kind="ExternalInput")
    w_q = nc.dram_tensor("w_q", [OUT_SH, IN], INT8, kind="ExternalInput")
    w_scales = nc.dram_tensor("w_scales", [OUT_SH, NB], FP32, kind="ExternalInput")
    bias = nc.dram_tensor("bias", [OUT_SH], FP32, kind="ExternalInput")
    ident = nc.dram_tensor("ident", [P, P], BF16, kind="ExternalInput")
    out = nc.dram_tensor("out", [TOK, OUT_SH], FP32, kind="ExternalOutput")
    do_gemm = mode not in ("prep",)
    stub_x = mode in ("gemm", "wonly", "gemm_nodrain", "gemm_nostore")
    stub_w = mode in ("gemm", "xonly", "gemm_nodrain", "gemm_nostore")
    do_drain = mode not in ("gemm_nodrain",)
    do_store = mode not in ("gemm_nodrain", "gemm_nostore")

    prev_last = None
    with tile.TileContext(nc) as tc:
      for _rep in range(repeats):
       with ExitStack() as ctx:
        const_pool = ctx.enter_context(tc.tile_pool(name=f"const{_rep}", bufs=1))
        wq_pool = ctx.enter_context(tc.tile_pool(name=f"wq{_rep}", bufs=2))
        wd_pool = ctx.enter_context(tc.tile_pool(name=f"wd{_rep}", bufs=2))
        wt_pool = ctx.enter_context(tc.tile_pool(name=f"wt{_rep}", bufs=1))
        xt_pool = ctx.enter_context(tc.tile_pool(name=f"xt{_rep}", bufs=10))
        out_pool = ctx.enter_context(tc.tile_pool(name=f"out{_rep}", bufs=4))
        psum_bufs = int(os.environ.get("PSUM_BUFS", "6"))
        psum_pool = ctx.enter_context(
            tc.tile_pool(name=f"psum{_rep}", bufs=psum_bufs, space="PSUM")
        )
        if psum_bufs >= 8:
            pst_pool = psum_pool
        else:
            pst_pool = ctx.enter_context(
                tc.tile_pool(name=f"pst{_rep}", bufs=2, space="PSUM")
            )

        entries = []

        idt = const_pool.tile([P, P], BF16, tag="idt")
        entries.append(nc.sync.dma_start(idt[:], ident.ap()[:, :]))

        sc_tiles = []
        for o in range(OT):
            sct = const_pool.tile([P, NB], FP32, tag=f"sc_{o}")
            entries.append(
                nc.sync.dma_start(sct[:], w_scales.ap()[o * P : (o + 1) * P, :])
            )
            sc_tiles.append(sct)

        bias_rep = const_pool.tile([P, OUT_SH], FP32, tag="bias_rep")
        entries.append(
            nc.sync.dma_start(
                bias_rep[:], bias.ap().unsqueeze(0).to_broadcast([P, OUT_SH])
            )
        )

        wt = wt_pool.tile([P, KT, OUT_SH], BF16, tag="wt")

        def pe_transpose(dst3, src2d_slices):
            """Transpose KT [128,128] bf16 slices into dst3 [128, KT, 128],
            batching TB per bf16 PSUM tile with one ACT drain each."""
            res = None
            for k0 in range(0, len(src2d_slices), TB):
                nb2 = min(TB, len(src2d_slices) - k0)
                pst = pst_pool.tile([P, TB * P], BF16, tag="pst")
                for j in range(nb2):
                    nc.tensor.matmul(
                        pst[:, j * P : (j + 1) * P],
                        src2d_slices[k0 + j],
                        idt[:],
                        is_transpose=True,
                        skip_group_check=True,
                    )
                res = nc.scalar.copy(
                    dst3[:, k0 : k0 + nb2, :],
                    pst[:, 0 : nb2 * P].rearrange("p (k q) -> p k q", q=P),
                )
            return res

        def w_chunk(o):
            rows = slice(o * P, (o + 1) * P)
            if stub_w:
                entries.append(
                    nc.sync.dma_start(wt[:, 0, o * P : (o + 1) * P], ident.ap()[:, :])
                )
                return
            wd = wd_pool.tile([P, IN], BF16, tag="wd")
            wq_i = wq_pool.tile([P, IN], INT8, tag="wq")
            entries.append(nc.sync.dma_start(wq_i[:], w_q.ap()[rows, :]))
            nc.vector.tensor_mul(
                wd[:].rearrange("p (b q) -> p b q", q=QK),
                wq_i[:].rearrange("p (b q) -> p b q", q=QK),
                sc_tiles[o][:].unsqueeze(2).to_broadcast([P, NB, QK]),
            )
            pe_transpose(
                wt[:, :, o * P : (o + 1) * P],
                [wd[:, k * P : (k + 1) * P] for k in range(KT)],
            )

        def load_xt(m):
            """DMA one [128, KT, 128] x-tile straight from the
            host-transposed xT (no on-device transpose needed)."""
            xt_m = xt_pool.tile([P, KT, P], BF16, tag="xt")
            if stub_x:
                entries.append(nc.sync.dma_start(xt_m[:, 0, :], ident.ap()[:, :]))
                return xt_m
            entries.append(
                nc.gpsimd.dma_start(
                    xt_m[:, :, :],
                    xt_d.ap()[:, m * P : (m + 1) * P].rearrange(
                        "(k p) j -> p k j", p=P
                    ),
                )
            )
            return xt_m

        def gemm_mgroup(xt_m, m, ns):
            if not do_gemm:
                return None
            tok0 = m * P
            pss = []
            for n in ns:
                ps_n = psum_pool.tile([P, NCOL], FP32, tag="ps")
                pss.append(ps_n)
            for k in range(KT):
                for i, n in enumerate(ns):
                    nc.tensor.matmul(
                        pss[i][:],
                        xt_m[:, k, :],
                        wt[:, k, n * NCOL : (n + 1) * NCOL],
                        start=(k == 0),
                        stop=(k == KT - 1),
                    )
            last = None
            if not do_drain:
                return None
            for i, n in enumerate(ns):
                ob = out_pool.tile([P, NCOL], FP32, tag="ob")
                nc.vector.tensor_add(
                    ob[:], pss[i][:], bias_rep[:, n * NCOL : (n + 1) * NCOL]
                )
                if do_store:
                    last = nc.scalar.dma_start(
                        out.ap()[tok0 : tok0 + P, n * NCOL : (n + 1) * NCOL], ob[:]
                    )
                else:
                    last = None
            return last

        # Head: w chunks (int8, small) + the first two x blocks; the GEMM's
        # k-inner/n-inner order staggers naturally against the arriving wt
        # columns (n=0 needs chunks 0-3 only, etc.), so no phase split.
        xt_tiles = {}
        for o in range(OT):
            w_chunk(o)
        for m in range(NB1 * MT):
            xt_tiles[m] = load_xt(m)

        # Main loop with one-block x prefetch.
        last_store = None
        for b in range(NBLK):
            for j in range(MT):
                nm = (b + 1) * MT + j
                if NB1 * MT <= nm < NBLK * MT:
                    xt_tiles[nm] = load_xt(nm)
            for j in range(MT):
                m = b * MT + j
                xt_m = xt_tiles.pop(m)
                last_store = gemm_mgroup(xt_m, m, list(range(NT)))

        if last_store is None:
            last_store = entries[-1]
        if serialize and prev_last is not None:
            for e in entries:
                add_dep_helper(e.ins, prev_last.ins, reason="serialize reps")
        prev_last = last_store
    return nc


def _dedup_ldweights(nc):
    """Drop InstLdweights that reload the exact weights already resident in
    the PE array (sync-free ones only). The legalizer emits one LDW per
    matmul with no dedup; k-inner/n-inner ordering makes 2/3 redundant."""
    removed = 0
    for fn in nc.m.functions:
        for bb in fn.blocks:
            insts = list(bb.instructions)
            new, last_fp = [], None
            for i in insts:
                if type(i).__name__ == "InstLdweights":
                    si = i.sync_info
                    clean = si is None or (
                        len(si.on_wait) == 0 and len(si.on_update) == 0
                    )
                    fp = str(i.ins[-1])
                    if clean and fp == last_fp:
                        removed += 1
                        continue
                    last_fp = fp
                new.append(i)
            if len(new) != len(insts):
                bb.instructions = new
    return removed


def _batch_pe_updates(nc):
    """Coalesce per-matmul sem-inc(+1) updates (Tile's pool reader
    tracking posts one on EVERY matmul, ~50ns each on HW): strip them and
    fold the counts into the last matmul before each InstLdweights / any
    PE-stream sem wait / block end. Totals are preserved and updates only
    move later, so no wait is satisfied early. The PE never blocks on its
    own pending counts because we flush before every PE instruction that
    waits."""
    moved = 0
    limit = int(os.environ.get("BATCH_LIMIT", "1000000"))
    nbatched = 0
    for fn in nc.m.functions:
        for bb in fn.blocks:
            run = []  # consecutive matmuls with strippable updates

            def end_run():
                nonlocal moved, nbatched
                if len(run) >= 2 and nbatched < limit:
                    nbatched += 1
                    last = run[-1]
                    last_by_id = {u.id: u for u in last.sync_info.on_update}
                    for mm in run[:-1]:
                        ups = mm.sync_info.on_update
                        if all(u.id in last_by_id for u in ups):
                            for u in ups:
                                tgt = last_by_id[u.id]
                                tgt.update_value = tgt.update_value + u.update_value
                                # 'sem-inc' is a fixed +1 tick whose value
                                # field is ignored; value-carrying increments
                                # must use 'sem-add-imm' (what then_inc emits)
                                tgt.update_mode = "sem-add-imm"
                            mm.sync_info.on_update = []
                            moved += 1
                run.clear()

            for i in bb.instructions:
                if not str(i.engine).endswith("PE"):
                    continue
                si = i.sync_info
                t = type(i).__name__
                if (
                    t == "InstMatmult"
                    and si is not None
                    and len(si.on_wait) == 0
                    and si.on_update
                    and all(
                        u.update_mode == "sem-inc" and u.update_value == 1
                        for u in si.on_update
                    )
                ):
                    run.append(i)
                elif t == "InstLdweights" and (si is None or len(si.on_wait) == 0):
                    pass  # waitless LDW is run-transparent (carries no updates)
                else:
                    end_run()
            end_run()
    return moved


BATCH_UPDATES = os.environ.get("BATCH_UPDATES", "1") == "1"


def _compile(nc):
    nc.compile()
    _dedup_ldweights(nc)
    if BATCH_UPDATES:
        _batch_pe_updates(nc)
    return nc


_COMPILED_NC = None


def _get_nc():
    global _COMPILED_NC
    if _COMPILED_NC is None:
        nc = bacc.Bacc("TRN2", target_bir_lowering=False, debug=False)
        _build(nc)
        _compile(nc)
        _COMPILED_NC = nc
    return _COMPILED_NC


def make_in_maps(inputs):
    import ml_dtypes

    x = np.asarray(inputs["x"], dtype=np.float32)
    xt = np.ascontiguousarray(x.T.astype(ml_dtypes.bfloat16))
    w_q = np.asarray(inputs["w_q"], dtype=np.int32).astype(np.int8)
    w_scales = np.asarray(inputs["w_scales"], dtype=np.float32)
    bias = np.asarray(inputs["bias"], dtype=np.float32)
    ident = np.eye(128, dtype=ml_dtypes.bfloat16)
    in_maps = []
    for c in range(N_CORES):
        r = slice(c * OUT_SH, (c + 1) * OUT_SH)
        in_maps.append(
            {
                "xt": xt,
                "w_q": np.ascontiguousarray(w_q[r]),
                "w_scales": np.ascontiguousarray(w_scales[r]),
                "bias": np.ascontiguousarray(bias[r]),
                "ident": ident,
            }
        )
    return in_maps


def kernel(x, w_q, w_scales, bias):
    assert x.shape == (TOK, IN) and w_q.shape == (OUT, IN)
    nc = _get_nc()
    in_maps = make_in_maps(
        {"x": x, "w_q": w_q, "w_scales": w_scales, "bias": bias}
    )
    res = run_bass_kernel_spmd(nc, in_maps, list(range(N_CORES)))
    return np.concatenate([res.results[c]["out"] for c in range(N_CORES)], axis=1)
